# revision 8
# baseline (speedup 1.0000x reference)
"""MACE message-passing layer on 8 Trainium2 NeuronCores — v2.

Graph-parallel / receiver-sharded, as the baseline, plus:
  - Host-side node relabeling balances receiver windows so every 128-node
    window holds <= 2048 edges: TW drops 18 -> 16 (256 edge tiles/core).
  - Split message msg = R.xs + (R.xs0).Y kept as two scatter matmuls into
    the same PSUM accumulator; the l=0 block of W2 is doubled host-side so
    the Y-term skips m=0 entirely (Y0 == 1).
  - The Y broadcast multiply runs in the DVE 2x bf16 mode by storing Y
    pair-duplicated ([..., m, 2]) so every operand's innermost AP dim is
    packed 2-byte.
  - h = silu(rad @ W1 + b1) computed inline per group with AF.Silu
    (PSUM -> bf16 SBUF in one ACT op); no DRAM bounce.
  - All matmul operands bf16 (fp32 moving operands cost 4 cycles/row).
"""
import os
import sys

sys.path.insert(0, '/opt/trn_rl_repo')

import numpy as np
import ml_dtypes

import json

import concourse.bass as bass
import concourse.mybir as mybir
import concourse.tile as tile


def _split_waits(bir_bytes, max_waits=1):
    """This container's walrus build only encodes one sync-wait command per
    instruction; hoist excess on_wait entries onto preceding Drain carriers."""
    bir = json.loads(bir_bytes)
    for func in bir['functions']:
        for blk in func['blocks']:
            insts = blk.get('instructions')
            if not insts:
                continue
            out = []
            for inst in insts:
                si = inst.get('sync_info')
                waits = (si or {}).get('on_wait') or []
                if len(waits) > max_waits and inst.get('engine') != 'Unassigned':
                    excess, keep = waits[:-max_waits], waits[-max_waits:]
                    for i in range(0, len(excess), max_waits):
                        out.append({
                            'debug': inst.get('debug', 0),
                            'engine': inst['engine'],
                            'ins': [], 'outs': [],
                            'is_reset_sema': False,
                            'name': f"{inst['name']}ws{i}",
                            'opcode': 'Drain',
                            'sync_info': {'on_update': [],
                                          'on_wait': excess[i:i + max_waits]},
                        })
                    si['on_wait'] = keep
                out.append(inst)
            blk['instructions'] = out
    return json.dumps(bir).encode()


def _install_compile_patch():
    import subprocess
    import concourse.bass_utils as bu
    import concourse.bass2jax as b2j
    if getattr(bu, "_mace_split_patch", False):
        return
    orig = bu.compile_bir_kernel

    def patched(bir_json, tmpdir, neff_name="file.neff"):
        return orig(_split_waits(bir_json), tmpdir, neff_name)

    bu.compile_bir_kernel = patched
    b2j.compile_bir_kernel = patched

    # let walrus dedupe consecutive LDWEIGHTS with identical stationary
    orig_check_call = subprocess.check_call

    def cc_patched(argv, *a, **kw):
        if (isinstance(argv, list) and argv
                and "walrus_driver" in str(argv[0])):
            argv = ["--enable-ldw-opt=true" if x == "--enable-ldw-opt=false"
                    else x for x in argv]
        return orig_check_call(argv, *a, **kw)

    subprocess.check_call = cc_patched
    bu._mace_split_patch = True


_install_compile_patch()

BF16 = mybir.dt.bfloat16
F32 = mybir.dt.float32
AF = mybir.ActivationFunctionType
ALU = mybir.AluOpType
nbf16 = ml_dtypes.bfloat16

# ---- problem constants (hardcoded per contest rules) ----
N_NODES = 16000
N_EDGES = 256000
F = 64
LM = 16
NRAD = 8
EPS = 0.25
# l-blocks over m=1..15 (mb = m-1): (mb offset, size, l)
LB = [(0, 3, 1), (3, 5, 2), (8, 7, 3)]
# full l-blocks over m=0..15 for the R.xs term
LA = [(0, 1, 0), (1, 3, 1), (4, 5, 2), (9, 7, 3)]

N_CORES = 8
WPC = 16                   # windows of 128 node-cols per core
NW = N_CORES * WPC         # 128 windows globally
NPW = 125                  # real nodes per window (125*128 = 16000)
TW = 16                    # edge tiles (x128) per window
TC = WPC * TW              # 256 tiles per core
EC = TC * 128              # 32768 edge slots per core
GT = 8                     # tiles per group
NG = TC // GT              # 32 groups (2 per window)
GE = GT * 128              # 1024 edges per group

_SQ3 = float(np.sqrt(3.0))
_SQ15 = float(np.sqrt(15.0))
_S5H = float(np.sqrt(5.0) / 2.0)
_C358 = float(np.sqrt(35.0 / 8.0))
_C105 = float(np.sqrt(105.0))
_C218 = float(np.sqrt(21.0 / 8.0))
_C7H = float(np.sqrt(7.0) / 2.0)


def build_program():
    nc = bass.Bass()

    nf_d = nc.declare_dram_parameter("nf", [N_NODES, LM * F], BF16, isOutput=False)
    snd_d = nc.declare_dram_parameter("snd", [128, TC], mybir.dt.int32, isOutput=False)
    oh_d = nc.declare_dram_parameter("ohm", [128, TC * 128], BF16, isOutput=False)
    vec_d = nc.declare_dram_parameter("vec", [128, TC * 3], F32, isOutput=False)
    rad_d = nc.declare_dram_parameter("rad", [NRAD, EC], BF16, isOutput=False)
    w1_d = nc.declare_dram_parameter("w1", [NRAD, F], BF16, isOutput=False)
    b1_d = nc.declare_dram_parameter("b1", [F, 1], F32, isOutput=False)
    w2_d = nc.declare_dram_parameter("w2", [F, 256], BF16, isOutput=False)
    wq_d = nc.declare_dram_parameter("wq", [128, F], F32, isOutput=False)
    c2_d = nc.declare_dram_parameter("c2w", [128, WPC * F], F32, isOutput=False)
    c3_d = nc.declare_dram_parameter("c3w", [128, WPC * F], F32, isOutput=False)
    nf0_d = nc.declare_dram_parameter("nf0", [128, WPC * F], F32, isOutput=False)
    u_d = nc.declare_dram_parameter("usc", [128, WPC * F], F32, isOutput=False)
    out_d = nc.declare_dram_parameter("out", [128, WPC], F32, isOutput=True)

    with tile.TileContext(nc) as tc:
        with (
            tc.tile_pool(name="const", bufs=1) as cpool,
            tc.tile_pool(name="ysc", bufs=1) as ypool,
            tc.tile_pool(name="rad", bufs=2) as radpool,
            tc.tile_pool(name="hps", bufs=2, space="PSUM") as hps,
            tc.tile_pool(name="rps", bufs=2, space="PSUM") as rps,
            tc.tile_pool(name="aggps", bufs=2, space="PSUM") as aggps,
            tc.tile_pool(name="xs", bufs=3) as xspool,
            tc.tile_pool(name="grp", bufs=2) as gpool,
            tc.tile_pool(name="msg", bufs=2) as mpool,
            tc.tile_pool(name="node", bufs=2) as npool,
        ):
            # ---------- constants in ----------
            w1_t = cpool.tile([NRAD, F], BF16)
            b1_t = cpool.tile([F, 1], F32)
            w2_t = cpool.tile([F, 256], BF16)
            wq_t = cpool.tile([128, F], F32)
            snd_t = cpool.tile([128, TC], mybir.dt.int32)
            vec_t = cpool.tile([128, TC * 3], F32)
            c2_t = cpool.tile([128, WPC * F], F32)
            c3_t = cpool.tile([128, WPC * F], F32)
            nf0_t = cpool.tile([128, WPC * F], F32)
            u_t = cpool.tile([128, WPC * F], F32)
            out_t = cpool.tile([128, WPC], F32)
            for t, d in [(w1_t, w1_d), (b1_t, b1_d), (w2_t, w2_d),
                         (wq_t, wq_d), (snd_t, snd_d),
                         (vec_t, vec_d), (c2_t, c2_d),
                         (c3_t, c3_d), (nf0_t, nf0_d), (u_t, u_d)]:
                nc.sync.dma_start(out=t[:], in_=d[:])

            # ---------- phase 1b: spherical harmonics, m=1..15, pair-dup ----------
            # y15[p, t, mb] bf16 (mb = m-1), then yrep[p, t, mb, 2].
            y15 = ypool.tile([128, TC * 15], BF16)
            y3 = y15[:].rearrange("p (t m) -> p t m", t=TC)
            v3 = vec_t[:].rearrange("p (t j) -> p t j", t=TC)
            x, y, z = v3[:, :, 0], v3[:, :, 1], v3[:, :, 2]
            sc = [ypool.tile([128, TC], F32, name=f"ysc{i}") for i in range(8)]
            x2, y2, z2, s, xy, d_, t_, u_ = sc
            nc.vector.tensor_tensor(x2[:], x, x, ALU.mult)
            nc.vector.tensor_tensor(y2[:], y, y, ALU.mult)
            nc.vector.tensor_tensor(z2[:], z, z, ALU.mult)
            nc.vector.tensor_tensor(s[:], x2[:], y2[:], ALU.add)
            nc.vector.tensor_tensor(s[:], s[:], z2[:], ALU.add)
            nc.vector.tensor_scalar_add(s[:], s[:], 1e-12)
            nc.scalar.activation(s[:], s[:], AF.Sqrt)        # r
            nc.vector.reciprocal(s[:], s[:])                 # 1/r
            nx, ny, nz = x2, y2, z2  # reuse scratch for normalized coords
            nc.vector.tensor_tensor(nx[:], x, s[:], ALU.mult)
            nc.vector.tensor_tensor(ny[:], y, s[:], ALU.mult)
            nc.vector.tensor_tensor(nz[:], z, s[:], ALU.mult)
            sx2, sy2, sz2 = s, xy, d_
            nc.vector.tensor_tensor(sx2[:], nx[:], nx[:], ALU.mult)
            nc.vector.tensor_tensor(sy2[:], ny[:], ny[:], ALU.mult)
            nc.vector.tensor_tensor(sz2[:], nz[:], nz[:], ALU.mult)
            nc.vector.tensor_scalar_mul(y3[:, :, 0], ny[:], _SQ3)
            nc.vector.tensor_scalar_mul(y3[:, :, 1], nz[:], _SQ3)
            nc.vector.tensor_scalar_mul(y3[:, :, 2], nx[:], _SQ3)
            nc.vector.scalar_tensor_tensor(y3[:, :, 3], nx[:], _SQ15, ny[:], ALU.mult, ALU.mult)
            nc.vector.scalar_tensor_tensor(y3[:, :, 4], ny[:], _SQ15, nz[:], ALU.mult, ALU.mult)
            nc.vector.tensor_scalar(y3[:, :, 5], sz2[:], 3.0 * _S5H, -_S5H, ALU.mult, ALU.add)
            nc.vector.scalar_tensor_tensor(y3[:, :, 6], nx[:], _SQ15, nz[:], ALU.mult, ALU.mult)
            nc.vector.tensor_tensor(t_[:], sx2[:], sy2[:], ALU.subtract)   # x2-y2
            nc.vector.tensor_scalar_mul(y3[:, :, 7], t_[:], _SQ15 / 2.0)
            nc.vector.scalar_tensor_tensor(y3[:, :, 13], t_[:], _C105 / 2.0, nz[:], ALU.mult, ALU.mult)
            nc.vector.tensor_scalar(u_[:], sx2[:], 3.0, None, ALU.mult)
            nc.vector.tensor_tensor(u_[:], u_[:], sy2[:], ALU.subtract)
            nc.vector.scalar_tensor_tensor(y3[:, :, 8], u_[:], _C358, ny[:], ALU.mult, ALU.mult)
            nc.vector.tensor_scalar(u_[:], sy2[:], 3.0, None, ALU.mult)
            nc.vector.tensor_tensor(u_[:], sx2[:], u_[:], ALU.subtract)
            nc.vector.scalar_tensor_tensor(y3[:, :, 14], u_[:], _C358, nx[:], ALU.mult, ALU.mult)
            nc.vector.tensor_tensor(u_[:], nx[:], ny[:], ALU.mult)
            nc.vector.scalar_tensor_tensor(y3[:, :, 9], u_[:], _C105, nz[:], ALU.mult, ALU.mult)
            nc.vector.tensor_scalar(u_[:], sz2[:], 5.0, -1.0, ALU.mult, ALU.add)
            nc.vector.scalar_tensor_tensor(y3[:, :, 10], u_[:], _C218, ny[:], ALU.mult, ALU.mult)
            nc.vector.scalar_tensor_tensor(y3[:, :, 12], u_[:], _C218, nx[:], ALU.mult, ALU.mult)
            nc.vector.tensor_scalar(u_[:], sz2[:], 5.0, -3.0, ALU.mult, ALU.add)
            nc.vector.scalar_tensor_tensor(y3[:, :, 11], u_[:], _C7H, nz[:], ALU.mult, ALU.mult)
            # pair-duplicate: yrep[p, t, mb, 2] (ACT copy; input broadcast on pair)
            yrep = ypool.tile([128, TC * 30], BF16)
            yr4 = yrep[:].rearrange("p (t m r) -> p t m r", t=TC, m=15)
            nc.scalar.activation(
                yr4,
                y3.unsqueeze(3).to_broadcast([128, TC, 15, 2]),
                AF.Copy)

            # ---------- phase 2: message passing ----------
            for w in range(WPC):
                agg = aggps.tile([128, LM * F], F32, space="PSUM")
                for gg in range(2):
                    g = w * 2 + gg
                    xs = xspool.tile([128, GT, LM * F], BF16)
                    for t in range(GT):
                        # one gather index per partition per call (HW limit)
                        nc.gpsimd.indirect_dma_start(
                            out=xs[:, t, :], out_offset=None, in_=nf_d[:],
                            in_offset=bass.IndirectOffsetOnAxis(
                                ap=snd_t[:, g * GT + t:g * GT + t + 1], axis=0))
                    oh = gpool.tile([128, GT * 128], BF16)
                    nc.sync.dma_start(out=oh[:], in_=oh_d[:, g * GT * 128:(g + 1) * GT * 128])
                    # h = silu(rad @ W1 + b1) inline, [64, GE] bf16
                    rad_ch = radpool.tile([NRAD, GE], BF16)
                    nc.sync.dma_start(out=rad_ch[:], in_=rad_d[:, g * GE:(g + 1) * GE])
                    h_g = gpool.tile([F, GE], BF16, tag="h_g")
                    for q in range(2):
                        hp = hps.tile([F, 512], F32, space="PSUM")
                        nc.tensor.matmul(hp[:], lhsT=w1_t[:],
                                         rhs=rad_ch[:, q * 512:(q + 1) * 512],
                                         start=True, stop=True)
                        nc.scalar.activation(h_g[:, q * 512:(q + 1) * 512], hp[:],
                                             AF.Silu, bias=b1_t[:], scale=1.0)
                    # R = h @ W2 per tile (l-major f-minor, l=0 doubled)
                    r_sb = gpool.tile([128, GT * 256], BF16)
                    for t in range(0, GT, 2):
                        rp = rps.tile([128, 512], F32, space="PSUM")
                        for k in range(2):
                            nc.tensor.matmul(rp[:, k * 256:(k + 1) * 256],
                                             lhsT=h_g[:, (t + k) * 128:(t + k + 1) * 128],
                                             rhs=w2_t[:], start=True, stop=True)
                        nc.scalar.activation(r_sb[:, t * 256:(t + 2) * 256], rp[:], AF.Copy)
                    r3 = r_sb[:].rearrange("p (t x) -> p t x", t=GT)
                    # b3 = R_{l>=1} * xs0  [p, t, 3*64]
                    b_sb = gpool.tile([128, GT * 192], BF16)
                    nc.vector.tensor_tensor(
                        b_sb[:].rearrange("p (t l f) -> p t l f", t=GT, l=3),
                        r3[:, :, F:4 * F].rearrange("p t (l f) -> p t l f", l=3),
                        xs[:, :, 0:F].unsqueeze(2).to_broadcast([128, GT, 3, F]),
                        ALU.mult)
                    b3 = b_sb[:].rearrange("p (t x) -> p t x", t=GT)
                    # mA = R_lm * xs  (all 16 m)
                    mA = mpool.tile([128, GT, LM * F], BF16)
                    for li, (off, sz, l) in enumerate(LA):
                        nc.vector.tensor_tensor(
                            mA[:, :, off * F:(off + sz) * F].rearrange("p t (m f) -> p t m f", m=sz),
                            xs[:, :, off * F:(off + sz) * F].rearrange("p t (m f) -> p t m f", m=sz),
                            r3[:, :, l * F:(l + 1) * F].unsqueeze(2).to_broadcast([128, GT, sz, F]),
                            ALU.mult)
                    # mB = b3_l * Y_m  (m=1..15), pair-packed iteration for 2x.
                    # Per-m ops keep every AP at partition + 3 free dims.
                    mB = mpool.tile([128, GT, 15 * F], BF16, tag="mB")
                    yg = yrep[:].rearrange("p (t m r) -> p t m r", t=TC, m=15)[
                        :, g * GT:(g + 1) * GT, :, :]
                    for (moff, sz, l) in LB:
                        for mi in range(moff, moff + sz):
                            nc.vector.tensor_tensor(
                                mB[:, :, mi * F:(mi + 1) * F].rearrange(
                                    "p t (f2 r) -> p t f2 r", r=2),
                                b3[:, :, (l - 1) * F:l * F].rearrange(
                                    "p t (f2 r) -> p t f2 r", r=2),
                                yg[:, :, mi, :].unsqueeze(2)
                                    .to_broadcast([128, GT, F // 2, 2]),
                                ALU.mult)
                    # scatter into agg PSUM: mA -> cols [0,1024), mB -> [64,1024)
                    for t in range(GT):
                        first = (gg == 0 and t == 0)
                        last = (gg == 1 and t == GT - 1)
                        lhs = oh[:, t * 128:(t + 1) * 128]
                        nc.tensor.matmul(agg[:, 0:512], lhsT=lhs,
                                         rhs=mA[:, t, 0:512],
                                         start=first, stop=False, skip_group_check=True)
                        nc.tensor.matmul(agg[:, 64:512], lhsT=lhs,
                                         rhs=mB[:, t, 0:448],
                                         start=False, stop=False, skip_group_check=True)
                        nc.tensor.matmul(agg[:, 512:1024], lhsT=lhs,
                                         rhs=mA[:, t, 512:1024],
                                         start=first, stop=False, skip_group_check=True)
                        nc.tensor.matmul(agg[:, 512:1024], lhsT=lhs,
                                         rhs=mB[:, t, 448:960],
                                         start=False, stop=last, skip_group_check=True)
                # ---------- node phase for window w ----------
                sq = npool.tile([128, LM * F], BF16, tag="sq")
                nc.scalar.activation(sq[:], agg[:], AF.Square)
                s8 = npool.tile([128, 8 * F], BF16, tag="s8")
                nc.vector.tensor_tensor(s8[:], sq[:, 0:8 * F], sq[:, 8 * F:16 * F], ALU.add)
                s4 = npool.tile([128, 4 * F], BF16, tag="s4")
                nc.vector.tensor_tensor(s4[:], s8[:, 0:4 * F], s8[:, 4 * F:8 * F], ALU.add)
                s2 = npool.tile([128, 2 * F], BF16, tag="s2")
                nc.vector.tensor_tensor(s2[:], s4[:, 0:2 * F], s4[:, 2 * F:4 * F], ALU.add)
                p2 = npool.tile([128, F], F32, tag="p2")
                nc.vector.tensor_tensor(p2[:], s2[:, 0:F], s2[:, F:2 * F], ALU.add)
                a0 = npool.tile([128, F], F32, tag="a0")
                nc.vector.tensor_copy(a0[:], agg[:, 0:F])
                t1 = npool.tile([128, F], F32, tag="t1")
                nc.vector.tensor_tensor(t1[:], p2[:], a0[:], ALU.mult)
                nc.vector.tensor_tensor(t1[:], t1[:], c3_t[:, w * F:(w + 1) * F], ALU.mult)
                t3 = npool.tile([128, F], F32, tag="t3")
                nc.vector.tensor_tensor(t3[:], p2[:], c2_t[:, w * F:(w + 1) * F], ALU.mult)
                gate = npool.tile([128, F], F32, tag="gate")
                nc.vector.scalar_tensor_tensor(gate[:], t3[:], 1.0, t1[:],
                                               ALU.add, ALU.add)
                q = npool.tile([128, F], F32, tag="q")
                nc.vector.tensor_tensor(q[:], a0[:], gate[:], ALU.mult)
                scr = npool.tile([128, F], F32, tag="scr")
                scr2 = npool.tile([128, F], F32, tag="scr2")
                nc.vector.tensor_tensor(scr[:], q[:], wq_t[:], ALU.mult)
                nc.vector.tensor_tensor(scr2[:], nf0_t[:, w * F:(w + 1) * F],
                                        u_t[:, w * F:(w + 1) * F], ALU.mult)
                nc.vector.tensor_tensor(scr[:], scr[:], scr2[:], ALU.add)
                nc.vector.tensor_reduce(out_t[:, w:w + 1], scr[:],
                                        mybir.AxisListType.X, ALU.add)

            nc.sync.dma_start(out=out_d[:], in_=out_t[:])
    return nc


def _balance_windows(receivers):
    """Assign nodes to NW windows (<=NPW nodes, <=TW*128 edges each).
    Returns (win_of_node, col_of_node)."""
    indeg = np.bincount(receivers, minlength=N_NODES)
    order = np.argsort(-indeg, kind="stable")
    loads = np.zeros(NW, np.int64)
    counts = np.zeros(NW, np.int64)
    win_of = np.zeros(N_NODES, np.int32)
    col_of = np.zeros(N_NODES, np.int32)
    # LPT greedy: heaviest node -> lightest feasible window
    import heapq
    heap = [(0, w) for w in range(NW)]
    heapq.heapify(heap)
    for nid in order:
        spill = []
        while True:
            load, w = heapq.heappop(heap)
            if counts[w] < NPW:
                break
            spill.append((load, w))
        for it in spill:
            heapq.heappush(heap, it)
        win_of[nid] = w
        col_of[nid] = counts[w]
        counts[w] += 1
        loads[w] += indeg[nid]
        heapq.heappush(heap, (int(loads[w]), w))
    assert loads.max() <= TW * 128, f"window overflow: {loads.max()}"
    return win_of, col_of


def host_prep(inputs):
    """Build the 8 per-core input maps + node permutation for assembly."""
    vectors = np.asarray(inputs["vectors"], np.float32)
    node_feats = np.asarray(inputs["node_feats"], np.float32)
    radial = np.asarray(inputs["radial_embedding"], np.float32)
    node_specie = np.asarray(inputs["node_specie"]).astype(np.int64)
    senders = np.asarray(inputs["senders"]).astype(np.int64)
    receivers = np.asarray(inputs["receivers"]).astype(np.int64)
    W_rad1 = np.asarray(inputs["W_rad1"], np.float32)
    b_rad1 = np.asarray(inputs["b_rad1"], np.float32)
    W_rad2 = np.asarray(inputs["W_rad2"], np.float32)
    W_skip = np.asarray(inputs["W_skip"], np.float32)
    c2 = np.asarray(inputs["c2"], np.float32)
    c3 = np.asarray(inputs["c3"], np.float32)
    W_out = np.asarray(inputs["W_out"], np.float32)

    win_of, col_of = _balance_windows(receivers)

    # shared tensors
    nf_g = np.ascontiguousarray(
        node_feats.transpose(0, 2, 1).reshape(N_NODES, LM * F)).astype(nbf16)
    w2lf = np.ascontiguousarray(
        W_rad2.reshape(F, F, 4).transpose(0, 2, 1).reshape(F, 4 * F)).astype(np.float32)
    w2lf[:, 0:F] *= 2.0  # fold the Y0==1 term into the l=0 block
    w2lf = w2lf.astype(nbf16)
    wq = np.tile((EPS * W_out[:, 0])[None, :], (128, 1)).astype(np.float32)
    u_sp = np.einsum('sfg,g->sf', W_skip[:, 0], W_out[:, 0])  # [10, F]
    U = u_sp[node_specie]                                     # [N, F]
    c2n = c2[node_specie] * (EPS ** 2)
    c3n = c3[node_specie] * (EPS ** 3)
    nf0 = node_feats[:, :, 0]                                 # [N, F]

    # per-(window,col) node tables
    def node_layout(arr_n, core):  # values per node -> [128, WPC*F]
        out = np.zeros((128, WPC, F), np.float32)
        sel = (win_of // WPC) == core
        w_l = (win_of[sel] % WPC)
        out[col_of[sel], w_l] = arr_n[sel]
        return np.ascontiguousarray(out.reshape(128, WPC * F))

    core_of_edge = win_of[receivers] // WPC
    win_l_of_edge = win_of[receivers] % WPC

    in_maps = []
    for c in range(N_CORES):
        snd_c = np.zeros(EC, np.int64)
        rcv_c = np.full(EC, 192.0, np.float32)
        vec_c = np.zeros((EC, 3), np.float32)
        rad_c = np.zeros((EC, NRAD), np.float32)
        for w in range(WPC):
            e_idx = np.nonzero((core_of_edge == c) & (win_l_of_edge == w))[0]
            ne = e_idx.size
            assert ne <= TW * 128, f"window overflow: core {c} win {w}: {ne}"
            base = w * TW * 128
            snd_c[base:base + ne] = senders[e_idx]
            rcv_c[base:base + ne] = col_of[receivers[e_idx]].astype(np.float32)
            vec_c[base:base + ne] = vectors[e_idx]
            rad_c[base:base + ne] = radial[e_idx]
        # host-built one-hot scatter matrices: oh[p, t*128 + j] =
        # (col[t*128+p] == j); pads (sentinel 192) never match.
        oh = (rcv_c.reshape(TC, 128).T[:, :, None]
              == np.arange(128, dtype=np.float32)[None, None, :])
        in_maps.append({
            "nf": nf_g,
            "snd": np.ascontiguousarray(
                snd_c.reshape(TC, 128).T.astype(np.int32)),
            "ohm": np.ascontiguousarray(oh.reshape(128, TC * 128)).astype(nbf16),
            "vec": np.ascontiguousarray(
                vec_c.reshape(TC, 128, 3).transpose(1, 0, 2).reshape(128, TC * 3)),
            "rad": np.ascontiguousarray(rad_c.T).astype(nbf16),
            "w1": W_rad1.astype(nbf16),
            "b1": b_rad1[:, None].astype(np.float32),
            "w2": w2lf,
            "wq": wq,
            "c2w": node_layout(c2n, c),
            "c3w": node_layout(c3n, c),
            "nf0": node_layout(nf0, c),
            "usc": node_layout(U, c),
        })
    return in_maps, win_of, col_of


def assemble_output(results, win_of, col_of):
    """results: list of 8 dicts with 'out' [128, WPC] -> [N_NODES, 1] f32."""
    full = np.zeros((N_NODES,), np.float32)
    outs = np.stack([np.asarray(results[c]["out"], np.float32)
                     for c in range(N_CORES)])  # [8, 128, WPC]
    full = outs[win_of // WPC, col_of, win_of % WPC]
    return full[:, None].copy()


_CACHED_NC = None
LAST_EXEC_NS = None
LAST_RESULTS = None


def kernel(**inputs):
    global _CACHED_NC, LAST_EXEC_NS, LAST_RESULTS
    from concourse.bass_utils import run_bass_kernel_spmd
    in_maps, win_of, col_of = host_prep(inputs)
    if _CACHED_NC is None:
        _CACHED_NC = build_program()
    trace = bool(int(os.environ.get("MACE_TRACE", "0")))
    kwargs = {}
    if trace:
        kwargs.update(trace=True, trace_cores=[0], tmpdir="/root/problem/trace_out")
        os.makedirs("/root/problem/trace_out", exist_ok=True)
    res = run_bass_kernel_spmd(_CACHED_NC, in_maps, list(range(N_CORES)), **kwargs)
    LAST_EXEC_NS = res.exec_time_ns
    LAST_RESULTS = res
    return assemble_output(res.results, win_of, col_of)


# revision 9
# speedup vs baseline: 1.0047x; 1.0047x over previous
"""MACE message-passing layer on 8 Trainium2 NeuronCores — v2.

Graph-parallel / receiver-sharded, as the baseline, plus:
  - Host-side node relabeling balances receiver windows so every 128-node
    window holds <= 2048 edges: TW drops 18 -> 16 (256 edge tiles/core).
  - Split message msg = R.xs + (R.xs0).Y kept as two scatter matmuls into
    the same PSUM accumulator; the l=0 block of W2 is doubled host-side so
    the Y-term skips m=0 entirely (Y0 == 1).
  - The Y broadcast multiply runs in the DVE 2x bf16 mode by storing Y
    pair-duplicated ([..., m, 2]) so every operand's innermost AP dim is
    packed 2-byte.
  - h = silu(rad @ W1 + b1) computed inline per group with AF.Silu
    (PSUM -> bf16 SBUF in one ACT op); no DRAM bounce.
  - All matmul operands bf16 (fp32 moving operands cost 4 cycles/row).
"""
import os
import sys

sys.path.insert(0, '/opt/trn_rl_repo')

import numpy as np
import ml_dtypes

import json

import concourse.bass as bass
import concourse.mybir as mybir
import concourse.tile as tile


def _split_waits(bir_bytes, max_waits=1):
    """This container's walrus build only encodes one sync-wait command per
    instruction; hoist excess on_wait entries onto preceding Drain carriers."""
    bir = json.loads(bir_bytes)
    for func in bir['functions']:
        for blk in func['blocks']:
            insts = blk.get('instructions')
            if not insts:
                continue
            out = []
            for inst in insts:
                si = inst.get('sync_info')
                waits = (si or {}).get('on_wait') or []
                if len(waits) > max_waits and inst.get('engine') != 'Unassigned':
                    excess, keep = waits[:-max_waits], waits[-max_waits:]
                    for i in range(0, len(excess), max_waits):
                        out.append({
                            'debug': inst.get('debug', 0),
                            'engine': inst['engine'],
                            'ins': [], 'outs': [],
                            'is_reset_sema': False,
                            'name': f"{inst['name']}ws{i}",
                            'opcode': 'Drain',
                            'sync_info': {'on_update': [],
                                          'on_wait': excess[i:i + max_waits]},
                        })
                    si['on_wait'] = keep
                out.append(inst)
            blk['instructions'] = out
    return json.dumps(bir).encode()


def _install_compile_patch():
    import subprocess
    import concourse.bass_utils as bu
    import concourse.bass2jax as b2j
    if getattr(bu, "_mace_split_patch", False):
        return
    orig = bu.compile_bir_kernel

    def patched(bir_json, tmpdir, neff_name="file.neff"):
        return orig(_split_waits(bir_json), tmpdir, neff_name)

    bu.compile_bir_kernel = patched
    b2j.compile_bir_kernel = patched

    # let walrus dedupe consecutive LDWEIGHTS with identical stationary
    orig_check_call = subprocess.check_call

    def cc_patched(argv, *a, **kw):
        if (isinstance(argv, list) and argv
                and "walrus_driver" in str(argv[0])):
            argv = ["--enable-ldw-opt=true" if x == "--enable-ldw-opt=false"
                    else x for x in argv]
        return orig_check_call(argv, *a, **kw)

    subprocess.check_call = cc_patched
    bu._mace_split_patch = True


_install_compile_patch()

BF16 = mybir.dt.bfloat16
F32 = mybir.dt.float32
AF = mybir.ActivationFunctionType
ALU = mybir.AluOpType
nbf16 = ml_dtypes.bfloat16

# ---- problem constants (hardcoded per contest rules) ----
N_NODES = 16000
N_EDGES = 256000
F = 64
LM = 16
NRAD = 8
EPS = 0.25
# l-blocks over m=1..15 (mb = m-1): (mb offset, size, l)
LB = [(0, 3, 1), (3, 5, 2), (8, 7, 3)]
# full l-blocks over m=0..15 for the R.xs term
LA = [(0, 1, 0), (1, 3, 1), (4, 5, 2), (9, 7, 3)]

N_CORES = 8
WPC = 16                   # windows of 128 node-cols per core
NW = N_CORES * WPC         # 128 windows globally
NPW = 125                  # real nodes per window (125*128 = 16000)
TW = 16                    # edge tiles (x128) per window
TC = WPC * TW              # 256 tiles per core
EC = TC * 128              # 32768 edge slots per core
GT = 8                     # tiles per group
NG = TC // GT              # 32 groups (2 per window)
GE = GT * 128              # 1024 edges per group

_SQ3 = float(np.sqrt(3.0))
_SQ15 = float(np.sqrt(15.0))
_S5H = float(np.sqrt(5.0) / 2.0)
_C358 = float(np.sqrt(35.0 / 8.0))
_C105 = float(np.sqrt(105.0))
_C218 = float(np.sqrt(21.0 / 8.0))
_C7H = float(np.sqrt(7.0) / 2.0)


def build_program():
    nc = bass.Bass()

    nf_d = nc.declare_dram_parameter("nf", [N_NODES, LM * F], mybir.dt.float8e4, isOutputFalse=False) if False else nc.declare_dram_parameter("nf", [N_NODES, LM * F], mybir.dt.float8e4, isOutput=False)
    snd_d = nc.declare_dram_parameter("snd", [128, TC], mybir.dt.int32, isOutput=False)
    oh_d = nc.declare_dram_parameter("ohm", [128, TC * 128], BF16, isOutput=False)
    vec_d = nc.declare_dram_parameter("vec", [128, TC * 3], F32, isOutput=False)
    rad_d = nc.declare_dram_parameter("rad", [NRAD, EC], BF16, isOutput=False)
    w1_d = nc.declare_dram_parameter("w1", [NRAD, F], BF16, isOutput=False)
    b1_d = nc.declare_dram_parameter("b1", [F, 1], F32, isOutput=False)
    w2_d = nc.declare_dram_parameter("w2", [F, 256], BF16, isOutput=False)
    wq_d = nc.declare_dram_parameter("wq", [128, F], F32, isOutput=False)
    c2_d = nc.declare_dram_parameter("c2w", [128, WPC * F], F32, isOutput=False)
    c3_d = nc.declare_dram_parameter("c3w", [128, WPC * F], F32, isOutput=False)
    nf0_d = nc.declare_dram_parameter("nf0", [128, WPC * F], F32, isOutput=False)
    u_d = nc.declare_dram_parameter("usc", [128, WPC * F], F32, isOutput=False)
    out_d = nc.declare_dram_parameter("out", [128, WPC], F32, isOutput=True)

    with tile.TileContext(nc) as tc:
        with (
            tc.tile_pool(name="const", bufs=1) as cpool,
            tc.tile_pool(name="ysc", bufs=1) as ypool,
            tc.tile_pool(name="rad", bufs=2) as radpool,
            tc.tile_pool(name="hps", bufs=2, space="PSUM") as hps,
            tc.tile_pool(name="rps", bufs=2, space="PSUM") as rps,
            tc.tile_pool(name="aggps", bufs=2, space="PSUM") as aggps,
            tc.tile_pool(name="xs", bufs=3) as xspool,
            tc.tile_pool(name="grp", bufs=2) as gpool,
            tc.tile_pool(name="msg", bufs=2) as mpool,
            tc.tile_pool(name="node", bufs=2) as npool,
        ):
            # ---------- constants in ----------
            w1_t = cpool.tile([NRAD, F], BF16)
            b1_t = cpool.tile([F, 1], F32)
            w2_t = cpool.tile([F, 256], BF16)
            wq_t = cpool.tile([128, F], F32)
            snd_t = cpool.tile([128, TC], mybir.dt.int32)
            vec_t = cpool.tile([128, TC * 3], F32)
            c2_t = cpool.tile([128, WPC * F], F32)
            c3_t = cpool.tile([128, WPC * F], F32)
            nf0_t = cpool.tile([128, WPC * F], F32)
            u_t = cpool.tile([128, WPC * F], F32)
            out_t = cpool.tile([128, WPC], F32)
            for t, d in [(w1_t, w1_d), (b1_t, b1_d), (w2_t, w2_d),
                         (wq_t, wq_d), (snd_t, snd_d),
                         (vec_t, vec_d), (c2_t, c2_d),
                         (c3_t, c3_d), (nf0_t, nf0_d), (u_t, u_d)]:
                nc.sync.dma_start(out=t[:], in_=d[:])

            # ---------- phase 1b: spherical harmonics, m=1..15, pair-dup ----------
            # y15[p, t, mb] bf16 (mb = m-1), then yrep[p, t, mb, 2].
            y15 = ypool.tile([128, TC * 15], BF16)
            y3 = y15[:].rearrange("p (t m) -> p t m", t=TC)
            v3 = vec_t[:].rearrange("p (t j) -> p t j", t=TC)
            x, y, z = v3[:, :, 0], v3[:, :, 1], v3[:, :, 2]
            sc = [ypool.tile([128, TC], F32, name=f"ysc{i}") for i in range(8)]
            x2, y2, z2, s, xy, d_, t_, u_ = sc
            nc.vector.tensor_tensor(x2[:], x, x, ALU.mult)
            nc.vector.tensor_tensor(y2[:], y, y, ALU.mult)
            nc.vector.tensor_tensor(z2[:], z, z, ALU.mult)
            nc.vector.tensor_tensor(s[:], x2[:], y2[:], ALU.add)
            nc.vector.tensor_tensor(s[:], s[:], z2[:], ALU.add)
            nc.vector.tensor_scalar_add(s[:], s[:], 1e-12)
            nc.scalar.activation(s[:], s[:], AF.Sqrt)        # r
            nc.vector.reciprocal(s[:], s[:])                 # 1/r
            nx, ny, nz = x2, y2, z2  # reuse scratch for normalized coords
            nc.vector.tensor_tensor(nx[:], x, s[:], ALU.mult)
            nc.vector.tensor_tensor(ny[:], y, s[:], ALU.mult)
            nc.vector.tensor_tensor(nz[:], z, s[:], ALU.mult)
            sx2, sy2, sz2 = s, xy, d_
            nc.vector.tensor_tensor(sx2[:], nx[:], nx[:], ALU.mult)
            nc.vector.tensor_tensor(sy2[:], ny[:], ny[:], ALU.mult)
            nc.vector.tensor_tensor(sz2[:], nz[:], nz[:], ALU.mult)
            nc.vector.tensor_scalar_mul(y3[:, :, 0], ny[:], _SQ3)
            nc.vector.tensor_scalar_mul(y3[:, :, 1], nz[:], _SQ3)
            nc.vector.tensor_scalar_mul(y3[:, :, 2], nx[:], _SQ3)
            nc.vector.scalar_tensor_tensor(y3[:, :, 3], nx[:], _SQ15, ny[:], ALU.mult, ALU.mult)
            nc.vector.scalar_tensor_tensor(y3[:, :, 4], ny[:], _SQ15, nz[:], ALU.mult, ALU.mult)
            nc.vector.tensor_scalar(y3[:, :, 5], sz2[:], 3.0 * _S5H, -_S5H, ALU.mult, ALU.add)
            nc.vector.scalar_tensor_tensor(y3[:, :, 6], nx[:], _SQ15, nz[:], ALU.mult, ALU.mult)
            nc.vector.tensor_tensor(t_[:], sx2[:], sy2[:], ALU.subtract)   # x2-y2
            nc.vector.tensor_scalar_mul(y3[:, :, 7], t_[:], _SQ15 / 2.0)
            nc.vector.scalar_tensor_tensor(y3[:, :, 13], t_[:], _C105 / 2.0, nz[:], ALU.mult, ALU.mult)
            nc.vector.tensor_scalar(u_[:], sx2[:], 3.0, None, ALU.mult)
            nc.vector.tensor_tensor(u_[:], u_[:], sy2[:], ALU.subtract)
            nc.vector.scalar_tensor_tensor(y3[:, :, 8], u_[:], _C358, ny[:], ALU.mult, ALU.mult)
            nc.vector.tensor_scalar(u_[:], sy2[:], 3.0, None, ALU.mult)
            nc.vector.tensor_tensor(u_[:], sx2[:], u_[:], ALU.subtract)
            nc.vector.scalar_tensor_tensor(y3[:, :, 14], u_[:], _C358, nx[:], ALU.mult, ALU.mult)
            nc.vector.tensor_tensor(u_[:], nx[:], ny[:], ALU.mult)
            nc.vector.scalar_tensor_tensor(y3[:, :, 9], u_[:], _C105, nz[:], ALU.mult, ALU.mult)
            nc.vector.tensor_scalar(u_[:], sz2[:], 5.0, -1.0, ALU.mult, ALU.add)
            nc.vector.scalar_tensor_tensor(y3[:, :, 10], u_[:], _C218, ny[:], ALU.mult, ALU.mult)
            nc.vector.scalar_tensor_tensor(y3[:, :, 12], u_[:], _C218, nx[:], ALU.mult, ALU.mult)
            nc.vector.tensor_scalar(u_[:], sz2[:], 5.0, -3.0, ALU.mult, ALU.add)
            nc.vector.scalar_tensor_tensor(y3[:, :, 11], u_[:], _C7H, nz[:], ALU.mult, ALU.mult)
            # pair-duplicate: yrep[p, t, mb, 2] (ACT copy; input broadcast on pair)
            yrep = ypool.tile([128, TC * 30], BF16)
            yr4 = yrep[:].rearrange("p (t m r) -> p t m r", t=TC, m=15)
            nc.scalar.activation(
                yr4,
                y3.unsqueeze(3).to_broadcast([128, TC, 15, 2]),
                AF.Copy)

            # ---------- phase 2: message passing ----------
            for w in range(WPC):
                agg = aggps.tile([128, LM * F], F32, space="PSUM")
                for gg in range(2):
                    g = w * 2 + gg
                    xs = xspool.tile([128, GT, LM * F], BF16)
                    for t in range(GT):
                        # one gather index per partition per call (HW limit)
                        nc.gpsimd.indirect_dma_start(
                            out=xs[:, t, :], out_offset=None, in_=nf_d[:],
                            in_offset=bass.IndirectOffsetOnAxis(
                                ap=snd_t[:, g * GT + t:g * GT + t + 1], axis=0))
                    oh = gpool.tile([128, GT * 128], BF16)
                    nc.sync.dma_start(out=oh[:], in_=oh_d[:, g * GT * 128:(g + 1) * GT * 128])
                    # h = silu(rad @ W1 + b1) inline, [64, GE] bf16
                    rad_ch = radpool.tile([NRAD, GE], BF16)
                    nc.sync.dma_start(out=rad_ch[:], in_=rad_d[:, g * GE:(g + 1) * GE])
                    h_g = gpool.tile([F, GE], BF16, tag="h_g")
                    for q in range(2):
                        hp = hps.tile([F, 512], F32, space="PSUM")
                        nc.tensor.matmul(hp[:], lhsT=w1_t[:],
                                         rhs=rad_ch[:, q * 512:(q + 1) * 512],
                                         start=True, stop=True)
                        nc.scalar.activation(h_g[:, q * 512:(q + 1) * 512], hp[:],
                                             AF.Silu, bias=b1_t[:], scale=1.0)
                    # R = h @ W2 per tile (l-major f-minor, l=0 doubled)
                    r_sb = gpool.tile([128, GT * 256], BF16)
                    for t in range(0, GT, 2):
                        rp = rps.tile([128, 512], F32, space="PSUM")
                        for k in range(2):
                            nc.tensor.matmul(rp[:, k * 256:(k + 1) * 256],
                                             lhsT=h_g[:, (t + k) * 128:(t + k + 1) * 128],
                                             rhs=w2_t[:], start=True, stop=True)
                        nc.scalar.activation(r_sb[:, t * 256:(t + 2) * 256], rp[:], AF.Copy)
                    r3 = r_sb[:].rearrange("p (t x) -> p t x", t=GT)
                    # b3 = R_{l>=1} * xs0  [p, t, 3*64]
                    b_sb = gpool.tile([128, GT * 192], BF16)
                    nc.vector.tensor_tensor(
                        b_sb[:].rearrange("p (t l f) -> p t l f", t=GT, l=3),
                        r3[:, :, F:4 * F].rearrange("p t (l f) -> p t l f", l=3),
                        xs[:, :, 0:F].unsqueeze(2).to_broadcast([128, GT, 3, F]),
                        ALU.mult)
                    b3 = b_sb[:].rearrange("p (t x) -> p t x", t=GT)
                    # mA = R_lm * xs  (all 16 m)
                    mA = mpool.tile([128, GT, LM * F], BF16)
                    for li, (off, sz, l) in enumerate(LA):
                        nc.vector.tensor_tensor(
                            mA[:, :, off * F:(off + sz) * F].rearrange("p t (m f) -> p t m f", m=sz),
                            xs[:, :, off * F:(off + sz) * F].rearrange("p t (m f) -> p t m f", m=sz),
                            r3[:, :, l * F:(l + 1) * F].unsqueeze(2).to_broadcast([128, GT, sz, F]),
                            ALU.mult)
                    # mB = b3_l * Y_m  (m=1..15), pair-packed iteration for 2x.
                    # Per-m ops keep every AP at partition + 3 free dims.
                    mB = mpool.tile([128, GT, 15 * F], BF16, tag="mB")
                    yg = yrep[:].rearrange("p (t m r) -> p t m r", t=TC, m=15)[
                        :, g * GT:(g + 1) * GT, :, :]
                    for (moff, sz, l) in LB:
                        for mi in range(moff, moff + sz):
                            nc.vector.tensor_tensor(
                                mB[:, :, mi * F:(mi + 1) * F].rearrange(
                                    "p t (f2 r) -> p t f2 r", r=2),
                                b3[:, :, (l - 1) * F:l * F].rearrange(
                                    "p t (f2 r) -> p t f2 r", r=2),
                                yg[:, :, mi, :].unsqueeze(2)
                                    .to_broadcast([128, GT, F // 2, 2]),
                                ALU.mult)
                    # scatter into agg PSUM: mA -> cols [0,1024), mB -> [64,1024)
                    for t in range(GT):
                        first = (gg == 0 and t == 0)
                        last = (gg == 1 and t == GT - 1)
                        lhs = oh[:, t * 128:(t + 1) * 128]
                        nc.tensor.matmul(agg[:, 0:512], lhsT=lhs,
                                         rhs=mA[:, t, 0:512],
                                         start=first, stop=False, skip_group_check=True)
                        nc.tensor.matmul(agg[:, 64:512], lhsT=lhs,
                                         rhs=mB[:, t, 0:448],
                                         start=False, stop=False, skip_group_check=True)
                        nc.tensor.matmul(agg[:, 512:1024], lhsT=lhs,
                                         rhs=mA[:, t, 512:1024],
                                         start=first, stop=False, skip_group_check=True)
                        nc.tensor.matmul(agg[:, 512:1024], lhsT=lhs,
                                         rhs=mB[:, t, 448:960],
                                         start=False, stop=last, skip_group_check=True)
                # ---------- node phase for window w ----------
                sq = npool.tile([128, LM * F], BF16, tag="sq")
                nc.scalar.activation(sq[:], agg[:], AF.Square)
                s8 = npool.tile([128, 8 * F], BF16, tag="s8")
                nc.vector.tensor_tensor(s8[:], sq[:, 0:8 * F], sq[:, 8 * F:16 * F], ALU.add)
                s4 = npool.tile([128, 4 * F], BF16, tag="s4")
                nc.vector.tensor_tensor(s4[:], s8[:, 0:4 * F], s8[:, 4 * F:8 * F], ALU.add)
                s2 = npool.tile([128, 2 * F], BF16, tag="s2")
                nc.vector.tensor_tensor(s2[:], s4[:, 0:2 * F], s4[:, 2 * F:4 * F], ALU.add)
                p2 = npool.tile([128, F], F32, tag="p2")
                nc.vector.tensor_tensor(p2[:], s2[:, 0:F], s2[:, F:2 * F], ALU.add)
                a0 = npool.tile([128, F], F32, tag="a0")
                nc.vector.tensor_copy(a0[:], agg[:, 0:F])
                t1 = npool.tile([128, F], F32, tag="t1")
                nc.vector.tensor_tensor(t1[:], p2[:], a0[:], ALU.mult)
                nc.vector.tensor_tensor(t1[:], t1[:], c3_t[:, w * F:(w + 1) * F], ALU.mult)
                t3 = npool.tile([128, F], F32, tag="t3")
                nc.vector.tensor_tensor(t3[:], p2[:], c2_t[:, w * F:(w + 1) * F], ALU.mult)
                gate = npool.tile([128, F], F32, tag="gate")
                nc.vector.scalar_tensor_tensor(gate[:], t3[:], 1.0, t1[:],
                                               ALU.add, ALU.add)
                q = npool.tile([128, F], F32, tag="q")
                nc.vector.tensor_tensor(q[:], a0[:], gate[:], ALU.mult)
                scr = npool.tile([128, F], F32, tag="scr")
                scr2 = npool.tile([128, F], F32, tag="scr2")
                nc.vector.tensor_tensor(scr[:], q[:], wq_t[:], ALU.mult)
                nc.vector.tensor_tensor(scr2[:], nf0_t[:, w * F:(w + 1) * F],
                                        u_t[:, w * F:(w + 1) * F], ALU.mult)
                nc.vector.tensor_tensor(scr[:], scr[:], scr2[:], ALU.add)
                nc.vector.tensor_reduce(out_t[:, w:w + 1], scr[:],
                                        mybir.AxisListType.X, ALU.add)

            nc.sync.dma_start(out=out_d[:], in_=out_t[:])
    return nc


def _balance_windows(receivers):
    """Assign nodes to NW windows (<=NPW nodes, <=TW*128 edges each).
    Returns (win_of_node, col_of_node)."""
    indeg = np.bincount(receivers, minlength=N_NODES)
    order = np.argsort(-indeg, kind="stable")
    loads = np.zeros(NW, np.int64)
    counts = np.zeros(NW, np.int64)
    win_of = np.zeros(N_NODES, np.int32)
    col_of = np.zeros(N_NODES, np.int32)
    # LPT greedy: heaviest node -> lightest feasible window
    import heapq
    heap = [(0, w) for w in range(NW)]
    heapq.heapify(heap)
    for nid in order:
        spill = []
        while True:
            load, w = heapq.heappop(heap)
            if counts[w] < NPW:
                break
            spill.append((load, w))
        for it in spill:
            heapq.heappush(heap, it)
        win_of[nid] = w
        col_of[nid] = counts[w]
        counts[w] += 1
        loads[w] += indeg[nid]
        heapq.heappush(heap, (int(loads[w]), w))
    assert loads.max() <= TW * 128, f"window overflow: {loads.max()}"
    return win_of, col_of


def host_prep(inputs):
    """Build the 8 per-core input maps + node permutation for assembly."""
    vectors = np.asarray(inputs["vectors"], np.float32)
    node_feats = np.asarray(inputs["node_feats"], np.float32)
    radial = np.asarray(inputs["radial_embedding"], np.float32)
    node_specie = np.asarray(inputs["node_specie"]).astype(np.int64)
    senders = np.asarray(inputs["senders"]).astype(np.int64)
    receivers = np.asarray(inputs["receivers"]).astype(np.int64)
    W_rad1 = np.asarray(inputs["W_rad1"], np.float32)
    b_rad1 = np.asarray(inputs["b_rad1"], np.float32)
    W_rad2 = np.asarray(inputs["W_rad2"], np.float32)
    W_skip = np.asarray(inputs["W_skip"], np.float32)
    c2 = np.asarray(inputs["c2"], np.float32)
    c3 = np.asarray(inputs["c3"], np.float32)
    W_out = np.asarray(inputs["W_out"], np.float32)

    win_of, col_of = _balance_windows(receivers)

    # shared tensors
    nf_g = np.ascontiguousarray(
        node_feats.transpose(0, 2, 1).reshape(N_NODES, LM * F)).astype(
            ml_dtypes.float8_e4m3fn).view(np.uint8)
    w2lf = np.ascontiguousarray(
        W_rad2.reshape(F, F, 4).transpose(0, 2, 1).reshape(F, 4 * F)).astype(np.float32)
    w2lf[:, 0:F] *= 2.0  # fold the Y0==1 term into the l=0 block
    w2lf = w2lf.astype(nbf16)
    wq = np.tile((EPS * W_out[:, 0])[None, :], (128, 1)).astype(np.float32)
    u_sp = np.einsum('sfg,g->sf', W_skip[:, 0], W_out[:, 0])  # [10, F]
    U = u_sp[node_specie]                                     # [N, F]
    c2n = c2[node_specie] * (EPS ** 2)
    c3n = c3[node_specie] * (EPS ** 3)
    nf0 = node_feats[:, :, 0]                                 # [N, F]

    # per-(window,col) node tables
    def node_layout(arr_n, core):  # values per node -> [128, WPC*F]
        out = np.zeros((128, WPC, F), np.float32)
        sel = (win_of // WPC) == core
        w_l = (win_of[sel] % WPC)
        out[col_of[sel], w_l] = arr_n[sel]
        return np.ascontiguousarray(out.reshape(128, WPC * F))

    core_of_edge = win_of[receivers] // WPC
    win_l_of_edge = win_of[receivers] % WPC

    in_maps = []
    for c in range(N_CORES):
        snd_c = np.zeros(EC, np.int64)
        rcv_c = np.full(EC, 192.0, np.float32)
        vec_c = np.zeros((EC, 3), np.float32)
        rad_c = np.zeros((EC, NRAD), np.float32)
        for w in range(WPC):
            e_idx = np.nonzero((core_of_edge == c) & (win_l_of_edge == w))[0]
            ne = e_idx.size
            assert ne <= TW * 128, f"window overflow: core {c} win {w}: {ne}"
            base = w * TW * 128
            snd_c[base:base + ne] = senders[e_idx]
            rcv_c[base:base + ne] = col_of[receivers[e_idx]].astype(np.float32)
            vec_c[base:base + ne] = vectors[e_idx]
            rad_c[base:base + ne] = radial[e_idx]
        # host-built one-hot scatter matrices: oh[p, t*128 + j] =
        # (col[t*128+p] == j); pads (sentinel 192) never match.
        oh = (rcv_c.reshape(TC, 128).T[:, :, None]
              == np.arange(128, dtype=np.float32)[None, None, :])
        in_maps.append({
            "nf": nf_g,
            "snd": np.ascontiguousarray(
                snd_c.reshape(TC, 128).T.astype(np.int32)),
            "ohm": np.ascontiguousarray(oh.reshape(128, TC * 128)).astype(nbf16),
            "vec": np.ascontiguousarray(
                vec_c.reshape(TC, 128, 3).transpose(1, 0, 2).reshape(128, TC * 3)),
            "rad": np.ascontiguousarray(rad_c.T).astype(nbf16),
            "w1": W_rad1.astype(nbf16),
            "b1": b_rad1[:, None].astype(np.float32),
            "w2": w2lf,
            "wq": wq,
            "c2w": node_layout(c2n, c),
            "c3w": node_layout(c3n, c),
            "nf0": node_layout(nf0, c),
            "usc": node_layout(U, c),
        })
    return in_maps, win_of, col_of


def assemble_output(results, win_of, col_of):
    """results: list of 8 dicts with 'out' [128, WPC] -> [N_NODES, 1] f32."""
    full = np.zeros((N_NODES,), np.float32)
    outs = np.stack([np.asarray(results[c]["out"], np.float32)
                     for c in range(N_CORES)])  # [8, 128, WPC]
    full = outs[win_of // WPC, col_of, win_of % WPC]
    return full[:, None].copy()


_CACHED_NC = None
LAST_EXEC_NS = None
LAST_RESULTS = None


def kernel(**inputs):
    global _CACHED_NC, LAST_EXEC_NS, LAST_RESULTS
    from concourse.bass_utils import run_bass_kernel_spmd
    in_maps, win_of, col_of = host_prep(inputs)
    if _CACHED_NC is None:
        _CACHED_NC = build_program()
    trace = bool(int(os.environ.get("MACE_TRACE", "0")))
    kwargs = {}
    if trace:
        kwargs.update(trace=True, trace_cores=[0], tmpdir="/root/problem/trace_out")
        os.makedirs("/root/problem/trace_out", exist_ok=True)
    res = run_bass_kernel_spmd(_CACHED_NC, in_maps, list(range(N_CORES)), **kwargs)
    LAST_EXEC_NS = res.exec_time_ns
    LAST_RESULTS = res
    return assemble_output(res.results, win_of, col_of)


# revision 10
# speedup vs baseline: 1.0256x; 1.0208x over previous
"""MACE message-passing layer on 8 Trainium2 NeuronCores — v2.

Graph-parallel / receiver-sharded, as the baseline, plus:
  - Host-side node relabeling balances receiver windows so every 128-node
    window holds <= 2048 edges: TW drops 18 -> 16 (256 edge tiles/core).
  - Split message msg = R.xs + (R.xs0).Y kept as two scatter matmuls into
    the same PSUM accumulator; the l=0 block of W2 is doubled host-side so
    the Y-term skips m=0 entirely (Y0 == 1).
  - The Y broadcast multiply runs in the DVE 2x bf16 mode by storing Y
    pair-duplicated ([..., m, 2]) so every operand's innermost AP dim is
    packed 2-byte.
  - h = silu(rad @ W1 + b1) computed inline per group with AF.Silu
    (PSUM -> bf16 SBUF in one ACT op); no DRAM bounce.
  - All matmul operands bf16 (fp32 moving operands cost 4 cycles/row).
"""
import os
import sys

sys.path.insert(0, '/opt/trn_rl_repo')

import numpy as np
import ml_dtypes

import json

import concourse.bass as bass
import concourse.mybir as mybir
import concourse.tile as tile


def _split_waits(bir_bytes, max_waits=1):
    """This container's walrus build only encodes one sync-wait command per
    instruction; hoist excess on_wait entries onto preceding Drain carriers."""
    bir = json.loads(bir_bytes)
    for func in bir['functions']:
        for blk in func['blocks']:
            insts = blk.get('instructions')
            if not insts:
                continue
            out = []
            for inst in insts:
                si = inst.get('sync_info')
                waits = (si or {}).get('on_wait') or []
                if len(waits) > max_waits and inst.get('engine') != 'Unassigned':
                    excess, keep = waits[:-max_waits], waits[-max_waits:]
                    for i in range(0, len(excess), max_waits):
                        out.append({
                            'debug': inst.get('debug', 0),
                            'engine': inst['engine'],
                            'ins': [], 'outs': [],
                            'is_reset_sema': False,
                            'name': f"{inst['name']}ws{i}",
                            'opcode': 'Drain',
                            'sync_info': {'on_update': [],
                                          'on_wait': excess[i:i + max_waits]},
                        })
                    si['on_wait'] = keep
                out.append(inst)
            blk['instructions'] = out
    return json.dumps(bir).encode()


def _install_compile_patch():
    import subprocess
    import concourse.bass_utils as bu
    import concourse.bass2jax as b2j
    if getattr(bu, "_mace_split_patch", False):
        return
    orig = bu.compile_bir_kernel

    def patched(bir_json, tmpdir, neff_name="file.neff"):
        return orig(_split_waits(bir_json), tmpdir, neff_name)

    bu.compile_bir_kernel = patched
    b2j.compile_bir_kernel = patched

    # let walrus dedupe consecutive LDWEIGHTS with identical stationary
    orig_check_call = subprocess.check_call

    def cc_patched(argv, *a, **kw):
        if (isinstance(argv, list) and argv
                and "walrus_driver" in str(argv[0])):
            argv = ["--enable-ldw-opt=true" if x == "--enable-ldw-opt=false"
                    else x for x in argv]
        return orig_check_call(argv, *a, **kw)

    subprocess.check_call = cc_patched
    bu._mace_split_patch = True


_install_compile_patch()

BF16 = mybir.dt.bfloat16
F32 = mybir.dt.float32
AF = mybir.ActivationFunctionType
ALU = mybir.AluOpType
nbf16 = ml_dtypes.bfloat16

# ---- problem constants (hardcoded per contest rules) ----
N_NODES = 16000
N_EDGES = 256000
F = 64
LM = 16
NRAD = 8
EPS = 0.25
# l-blocks over m=1..15 (mb = m-1): (mb offset, size, l)
LB = [(0, 3, 1), (3, 5, 2), (8, 7, 3)]
# full l-blocks over m=0..15 for the R.xs term
LA = [(0, 1, 0), (1, 3, 1), (4, 5, 2), (9, 7, 3)]

N_CORES = 8
WPC = 16                   # windows of 128 node-cols per core
NW = N_CORES * WPC         # 128 windows globally
NPW = 125                  # real nodes per window (125*128 = 16000)
TW = 16                    # edge tiles (x128) per window
TC = WPC * TW              # 256 tiles per core
EC = TC * 128              # 32768 edge slots per core
GT = 8                     # tiles per group
NG = TC // GT              # 32 groups (2 per window)
GE = GT * 128              # 1024 edges per group

_SQ3 = float(np.sqrt(3.0))
_SQ15 = float(np.sqrt(15.0))
_S5H = float(np.sqrt(5.0) / 2.0)
_C358 = float(np.sqrt(35.0 / 8.0))
_C105 = float(np.sqrt(105.0))
_C218 = float(np.sqrt(21.0 / 8.0))
_C7H = float(np.sqrt(7.0) / 2.0)


def build_program():
    nc = bass.Bass()

    nf_d = nc.declare_dram_parameter("nf", [N_NODES, LM * F], BF16, isOutput=False)
    snd_d = nc.declare_dram_parameter("snd", [128, TC], mybir.dt.int32, isOutput=False)
    oh_d = nc.declare_dram_parameter("ohm", [128, TC * 128], BF16, isOutput=False)
    vec_d = nc.declare_dram_parameter("vec", [128, TC * 3], F32, isOutput=False)
    rad_d = nc.declare_dram_parameter("rad", [NRAD, EC], BF16, isOutput=False)
    w1_d = nc.declare_dram_parameter("w1", [NRAD, F], BF16, isOutput=False)
    b1_d = nc.declare_dram_parameter("b1", [F, 1], F32, isOutput=False)
    w2_d = nc.declare_dram_parameter("w2", [F, 256], BF16, isOutput=False)
    wq_d = nc.declare_dram_parameter("wq", [128, F], F32, isOutput=False)
    c2_d = nc.declare_dram_parameter("c2w", [128, WPC * F], F32, isOutput=False)
    c3_d = nc.declare_dram_parameter("c3w", [128, WPC * F], F32, isOutput=False)
    nf0_d = nc.declare_dram_parameter("nf0", [128, WPC * F], F32, isOutput=False)
    u_d = nc.declare_dram_parameter("usc", [128, WPC * F], F32, isOutput=False)
    out_d = nc.declare_dram_parameter("out", [128, WPC], F32, isOutput=True)

    with tile.TileContext(nc) as tc:
        with (
            tc.tile_pool(name="const", bufs=1) as cpool,
            tc.tile_pool(name="ysc", bufs=1) as ypool,
            tc.tile_pool(name="rad", bufs=2) as radpool,
            tc.tile_pool(name="hps", bufs=2, space="PSUM") as hps,
            tc.tile_pool(name="rps", bufs=2, space="PSUM") as rps,
            tc.tile_pool(name="aggps", bufs=2, space="PSUM") as aggps,
            tc.tile_pool(name="xs", bufs=3) as xspool,
            tc.tile_pool(name="grp", bufs=2) as gpool,
            tc.tile_pool(name="msg", bufs=2) as mpool,
            tc.tile_pool(name="node", bufs=2) as npool,
        ):
            # ---------- constants in ----------
            w1_t = cpool.tile([NRAD, F], BF16)
            b1_t = cpool.tile([F, 1], F32)
            w2_t = cpool.tile([F, 256], BF16)
            wq_t = cpool.tile([128, F], F32)
            snd_t = cpool.tile([128, TC], mybir.dt.int32)
            vec_t = cpool.tile([128, TC * 3], F32)
            c2_t = cpool.tile([128, WPC * F], F32)
            c3_t = cpool.tile([128, WPC * F], F32)
            nf0_t = cpool.tile([128, WPC * F], F32)
            u_t = cpool.tile([128, WPC * F], F32)
            out_t = cpool.tile([128, WPC], F32)
            for t, d in [(w1_t, w1_d), (b1_t, b1_d), (w2_t, w2_d),
                         (wq_t, wq_d), (snd_t, snd_d),
                         (vec_t, vec_d), (c2_t, c2_d),
                         (c3_t, c3_d), (nf0_t, nf0_d), (u_t, u_d)]:
                nc.sync.dma_start(out=t[:], in_=d[:])

            # ---------- phase 1b: spherical harmonics, m=1..15, pair-dup ----------
            # y15[p, t, mb] bf16 (mb = m-1), then yrep[p, t, mb, 2].
            y15 = ypool.tile([128, TC * 15], BF16)
            y3 = y15[:].rearrange("p (t m) -> p t m", t=TC)
            v3 = vec_t[:].rearrange("p (t j) -> p t j", t=TC)
            x, y, z = v3[:, :, 0], v3[:, :, 1], v3[:, :, 2]
            sc = [ypool.tile([128, TC], F32, name=f"ysc{i}") for i in range(8)]
            x2, y2, z2, s, xy, d_, t_, u_ = sc
            nc.vector.tensor_tensor(x2[:], x, x, ALU.mult)
            nc.vector.tensor_tensor(y2[:], y, y, ALU.mult)
            nc.vector.tensor_tensor(z2[:], z, z, ALU.mult)
            nc.vector.tensor_tensor(s[:], x2[:], y2[:], ALU.add)
            nc.vector.tensor_tensor(s[:], s[:], z2[:], ALU.add)
            nc.vector.tensor_scalar_add(s[:], s[:], 1e-12)
            nc.scalar.activation(s[:], s[:], AF.Sqrt)        # r
            nc.vector.reciprocal(s[:], s[:])                 # 1/r
            nx, ny, nz = x2, y2, z2  # reuse scratch for normalized coords
            nc.vector.tensor_tensor(nx[:], x, s[:], ALU.mult)
            nc.vector.tensor_tensor(ny[:], y, s[:], ALU.mult)
            nc.vector.tensor_tensor(nz[:], z, s[:], ALU.mult)
            sx2, sy2, sz2 = s, xy, d_
            nc.vector.tensor_tensor(sx2[:], nx[:], nx[:], ALU.mult)
            nc.vector.tensor_tensor(sy2[:], ny[:], ny[:], ALU.mult)
            nc.vector.tensor_tensor(sz2[:], nz[:], nz[:], ALU.mult)
            nc.vector.tensor_scalar_mul(y3[:, :, 0], ny[:], _SQ3)
            nc.vector.tensor_scalar_mul(y3[:, :, 1], nz[:], _SQ3)
            nc.vector.tensor_scalar_mul(y3[:, :, 2], nx[:], _SQ3)
            nc.vector.scalar_tensor_tensor(y3[:, :, 3], nx[:], _SQ15, ny[:], ALU.mult, ALU.mult)
            nc.vector.scalar_tensor_tensor(y3[:, :, 4], ny[:], _SQ15, nz[:], ALU.mult, ALU.mult)
            nc.vector.tensor_scalar(y3[:, :, 5], sz2[:], 3.0 * _S5H, -_S5H, ALU.mult, ALU.add)
            nc.vector.scalar_tensor_tensor(y3[:, :, 6], nx[:], _SQ15, nz[:], ALU.mult, ALU.mult)
            nc.vector.tensor_tensor(t_[:], sx2[:], sy2[:], ALU.subtract)   # x2-y2
            nc.vector.tensor_scalar_mul(y3[:, :, 7], t_[:], _SQ15 / 2.0)
            nc.vector.scalar_tensor_tensor(y3[:, :, 13], t_[:], _C105 / 2.0, nz[:], ALU.mult, ALU.mult)
            nc.vector.tensor_scalar(u_[:], sx2[:], 3.0, None, ALU.mult)
            nc.vector.tensor_tensor(u_[:], u_[:], sy2[:], ALU.subtract)
            nc.vector.scalar_tensor_tensor(y3[:, :, 8], u_[:], _C358, ny[:], ALU.mult, ALU.mult)
            nc.vector.tensor_scalar(u_[:], sy2[:], 3.0, None, ALU.mult)
            nc.vector.tensor_tensor(u_[:], sx2[:], u_[:], ALU.subtract)
            nc.vector.scalar_tensor_tensor(y3[:, :, 14], u_[:], _C358, nx[:], ALU.mult, ALU.mult)
            nc.vector.tensor_tensor(u_[:], nx[:], ny[:], ALU.mult)
            nc.vector.scalar_tensor_tensor(y3[:, :, 9], u_[:], _C105, nz[:], ALU.mult, ALU.mult)
            nc.vector.tensor_scalar(u_[:], sz2[:], 5.0, -1.0, ALU.mult, ALU.add)
            nc.vector.scalar_tensor_tensor(y3[:, :, 10], u_[:], _C218, ny[:], ALU.mult, ALU.mult)
            nc.vector.scalar_tensor_tensor(y3[:, :, 12], u_[:], _C218, nx[:], ALU.mult, ALU.mult)
            nc.vector.tensor_scalar(u_[:], sz2[:], 5.0, -3.0, ALU.mult, ALU.add)
            nc.vector.scalar_tensor_tensor(y3[:, :, 11], u_[:], _C7H, nz[:], ALU.mult, ALU.mult)
            # pair-duplicate: yrep[p, t, mb, 2] (ACT copy; input broadcast on pair)
            yrep = ypool.tile([128, TC * 30], BF16)
            yr4 = yrep[:].rearrange("p (t m r) -> p t m r", t=TC, m=15)
            nc.scalar.activation(
                yr4,
                y3.unsqueeze(3).to_broadcast([128, TC, 15, 2]),
                AF.Copy)

            # ---------- phase 2: message passing ----------
            for w in range(WPC):
                agg = aggps.tile([128, LM * F], F32, space="PSUM")
                for gg in range(2):
                    g = w * 2 + gg
                    xs = xspool.tile([128, GT, LM * F], BF16)
                    for t in range(GT):
                        # one gather index per partition per call (HW limit)
                        nc.gpsimd.indirect_dma_start(
                            out=xs[:, t, :], out_offset=None, in_=nf_d[:],
                            in_offset=bass.IndirectOffsetOnAxis(
                                ap=snd_t[:, g * GT + t:g * GT + t + 1], axis=0))
                    oh = gpool.tile([128, GT * 128], BF16)
                    nc.sync.dma_start(out=oh[:], in_=oh_d[:, g * GT * 128:(g + 1) * GT * 128])
                    # h = silu(rad @ W1 + b1) inline, [64, GE] bf16
                    rad_ch = radpool.tile([NRAD, GE], BF16)
                    nc.sync.dma_start(out=rad_ch[:], in_=rad_d[:, g * GE:(g + 1) * GE])
                    h_g = gpool.tile([F, GE], BF16, tag="h_g")
                    for q in range(2):
                        hp = hps.tile([F, 512], F32, space="PSUM")
                        nc.tensor.matmul(hp[:], lhsT=w1_t[:],
                                         rhs=rad_ch[:, q * 512:(q + 1) * 512],
                                         start=True, stop=True)
                        nc.scalar.activation(h_g[:, q * 512:(q + 1) * 512], hp[:],
                                             AF.Silu, bias=b1_t[:], scale=1.0)
                    # R = h @ W2 per tile (l-major f-minor, l=0 doubled)
                    r_sb = gpool.tile([128, GT * 256], BF16)
                    for t in range(0, GT, 2):
                        rp = rps.tile([128, 512], F32, space="PSUM")
                        for k in range(2):
                            nc.tensor.matmul(rp[:, k * 256:(k + 1) * 256],
                                             lhsT=h_g[:, (t + k) * 128:(t + k + 1) * 128],
                                             rhs=w2_t[:], start=True, stop=True)
                        nc.scalar.activation(r_sb[:, t * 256:(t + 2) * 256], rp[:], AF.Copy)
                    r3 = r_sb[:].rearrange("p (t x) -> p t x", t=GT)
                    # b3 = R_{l>=1} * xs0  [p, t, 3*64]
                    b_sb = gpool.tile([128, GT * 192], BF16)
                    nc.vector.tensor_tensor(
                        b_sb[:].rearrange("p (t l f) -> p t l f", t=GT, l=3),
                        r3[:, :, F:4 * F].rearrange("p t (l f) -> p t l f", l=3),
                        xs[:, :, 0:F].unsqueeze(2).to_broadcast([128, GT, 3, F]),
                        ALU.mult)
                    b3 = b_sb[:].rearrange("p (t x) -> p t x", t=GT)
                    # mA = R_lm * xs  (all 16 m)
                    mA = mpool.tile([128, GT, LM * F], BF16)
                    for li, (off, sz, l) in enumerate(LA):
                        nc.vector.tensor_tensor(
                            mA[:, :, off * F:(off + sz) * F].rearrange("p t (m f) -> p t m f", m=sz),
                            xs[:, :, off * F:(off + sz) * F].rearrange("p t (m f) -> p t m f", m=sz),
                            r3[:, :, l * F:(l + 1) * F].unsqueeze(2).to_broadcast([128, GT, sz, F]),
                            ALU.mult)
                    # mB = b3_l * Y_m  (m=1..15), pair-packed iteration for 2x.
                    # Per-m ops keep every AP at partition + 3 free dims.
                    mB = mpool.tile([128, GT, 15 * F], BF16, tag="mB")
                    yg = yrep[:].rearrange("p (t m r) -> p t m r", t=TC, m=15)[
                        :, g * GT:(g + 1) * GT, :, :]
                    for (moff, sz, l) in LB:
                        for mi in range(moff, moff + sz):
                            nc.vector.tensor_tensor(
                                mB[:, :, mi * F:(mi + 1) * F].rearrange(
                                    "p t (f2 r) -> p t f2 r", r=2),
                                b3[:, :, (l - 1) * F:l * F].rearrange(
                                    "p t (f2 r) -> p t f2 r", r=2),
                                yg[:, :, mi, :].unsqueeze(2)
                                    .to_broadcast([128, GT, F // 2, 2]),
                                ALU.mult)
                    # scatter into agg PSUM: mA -> cols [0,1024), mB -> [64,1024)
                    for t in range(GT):
                        first = (gg == 0 and t == 0)
                        last = (gg == 1 and t == GT - 1)
                        lhs = oh[:, t * 128:(t + 1) * 128]
                        nc.tensor.matmul(agg[:, 0:512], lhsT=lhs,
                                         rhs=mA[:, t, 0:512],
                                         start=first, stop=False, skip_group_check=True)
                        nc.tensor.matmul(agg[:, 64:512], lhsT=lhs,
                                         rhs=mB[:, t, 0:448],
                                         start=False, stop=False, skip_group_check=True)
                        nc.tensor.matmul(agg[:, 512:1024], lhsT=lhs,
                                         rhs=mA[:, t, 512:1024],
                                         start=first, stop=False, skip_group_check=True)
                        nc.tensor.matmul(agg[:, 512:1024], lhsT=lhs,
                                         rhs=mB[:, t, 448:960],
                                         start=False, stop=last, skip_group_check=True)
                # ---------- node phase for window w ----------
                sq = npool.tile([128, LM * F], BF16, tag="sq")
                nc.scalar.activation(sq[:], agg[:], AF.Square)
                s8 = npool.tile([128, 8 * F], BF16, tag="s8")
                nc.vector.tensor_tensor(s8[:], sq[:, 0:8 * F], sq[:, 8 * F:16 * F], ALU.add)
                s4 = npool.tile([128, 4 * F], BF16, tag="s4")
                nc.vector.tensor_tensor(s4[:], s8[:, 0:4 * F], s8[:, 4 * F:8 * F], ALU.add)
                s2 = npool.tile([128, 2 * F], BF16, tag="s2")
                nc.vector.tensor_tensor(s2[:], s4[:, 0:2 * F], s4[:, 2 * F:4 * F], ALU.add)
                p2 = npool.tile([128, F], F32, tag="p2")
                nc.vector.tensor_tensor(p2[:], s2[:, 0:F], s2[:, F:2 * F], ALU.add)
                a0 = npool.tile([128, F], F32, tag="a0")
                nc.vector.tensor_copy(a0[:], agg[:, 0:F])
                t1 = npool.tile([128, F], F32, tag="t1")
                nc.vector.tensor_tensor(t1[:], p2[:], a0[:], ALU.mult)
                nc.vector.tensor_tensor(t1[:], t1[:], c3_t[:, w * F:(w + 1) * F], ALU.mult)
                t3 = npool.tile([128, F], F32, tag="t3")
                nc.vector.tensor_tensor(t3[:], p2[:], c2_t[:, w * F:(w + 1) * F], ALU.mult)
                gate = npool.tile([128, F], F32, tag="gate")
                nc.vector.scalar_tensor_tensor(gate[:], t3[:], 1.0, t1[:],
                                               ALU.add, ALU.add)
                q = npool.tile([128, F], F32, tag="q")
                nc.vector.tensor_tensor(q[:], a0[:], gate[:], ALU.mult)
                scr = npool.tile([128, F], F32, tag="scr")
                scr2 = npool.tile([128, F], F32, tag="scr2")
                nc.vector.tensor_tensor(scr[:], q[:], wq_t[:], ALU.mult)
                nc.vector.tensor_tensor(scr2[:], nf0_t[:, w * F:(w + 1) * F],
                                        u_t[:, w * F:(w + 1) * F], ALU.mult)
                nc.vector.tensor_tensor(scr[:], scr[:], scr2[:], ALU.add)
                nc.vector.tensor_reduce(out_t[:, w:w + 1], scr[:],
                                        mybir.AxisListType.X, ALU.add)

            nc.sync.dma_start(out=out_d[:], in_=out_t[:])
    return nc


def _balance_windows(receivers):
    """Assign nodes to NW windows (<=NPW nodes, <=TW*128 edges each).
    Returns (win_of_node, col_of_node)."""
    indeg = np.bincount(receivers, minlength=N_NODES)
    order = np.argsort(-indeg, kind="stable")
    loads = np.zeros(NW, np.int64)
    counts = np.zeros(NW, np.int64)
    win_of = np.zeros(N_NODES, np.int32)
    col_of = np.zeros(N_NODES, np.int32)
    # LPT greedy: heaviest node -> lightest feasible window
    import heapq
    heap = [(0, w) for w in range(NW)]
    heapq.heapify(heap)
    for nid in order:
        spill = []
        while True:
            load, w = heapq.heappop(heap)
            if counts[w] < NPW:
                break
            spill.append((load, w))
        for it in spill:
            heapq.heappush(heap, it)
        win_of[nid] = w
        col_of[nid] = counts[w]
        counts[w] += 1
        loads[w] += indeg[nid]
        heapq.heappush(heap, (int(loads[w]), w))
    assert loads.max() <= TW * 128, f"window overflow: {loads.max()}"
    return win_of, col_of


def host_prep(inputs):
    """Build the 8 per-core input maps + node permutation for assembly."""
    vectors = np.asarray(inputs["vectors"], np.float32)
    node_feats = np.asarray(inputs["node_feats"], np.float32)
    radial = np.asarray(inputs["radial_embedding"], np.float32)
    node_specie = np.asarray(inputs["node_specie"]).astype(np.int64)
    senders = np.asarray(inputs["senders"]).astype(np.int64)
    receivers = np.asarray(inputs["receivers"]).astype(np.int64)
    W_rad1 = np.asarray(inputs["W_rad1"], np.float32)
    b_rad1 = np.asarray(inputs["b_rad1"], np.float32)
    W_rad2 = np.asarray(inputs["W_rad2"], np.float32)
    W_skip = np.asarray(inputs["W_skip"], np.float32)
    c2 = np.asarray(inputs["c2"], np.float32)
    c3 = np.asarray(inputs["c3"], np.float32)
    W_out = np.asarray(inputs["W_out"], np.float32)

    win_of, col_of = _balance_windows(receivers)

    # shared tensors
    nf_g = np.ascontiguousarray(
        node_feats.transpose(0, 2, 1).reshape(N_NODES, LM * F)).astype(nbf16)
    w2lf = np.ascontiguousarray(
        W_rad2.reshape(F, F, 4).transpose(0, 2, 1).reshape(F, 4 * F)).astype(np.float32)
    w2lf[:, 0:F] *= 2.0  # fold the Y0==1 term into the l=0 block
    w2lf = w2lf.astype(nbf16)
    wq = np.tile((EPS * W_out[:, 0])[None, :], (128, 1)).astype(np.float32)
    u_sp = np.einsum('sfg,g->sf', W_skip[:, 0], W_out[:, 0])  # [10, F]
    U = u_sp[node_specie]                                     # [N, F]
    c2n = c2[node_specie] * (EPS ** 2)
    c3n = c3[node_specie] * (EPS ** 3)
    nf0 = node_feats[:, :, 0]                                 # [N, F]

    # per-(window,col) node tables
    def node_layout(arr_n, core):  # values per node -> [128, WPC*F]
        out = np.zeros((128, WPC, F), np.float32)
        sel = (win_of // WPC) == core
        w_l = (win_of[sel] % WPC)
        out[col_of[sel], w_l] = arr_n[sel]
        return np.ascontiguousarray(out.reshape(128, WPC * F))

    core_of_edge = win_of[receivers] // WPC
    win_l_of_edge = win_of[receivers] % WPC

    in_maps = []
    for c in range(N_CORES):
        snd_c = np.zeros(EC, np.int64)
        rcv_c = np.full(EC, 192.0, np.float32)
        vec_c = np.zeros((EC, 3), np.float32)
        rad_c = np.zeros((EC, NRAD), np.float32)
        for w in range(WPC):
            e_idx = np.nonzero((core_of_edge == c) & (win_l_of_edge == w))[0]
            ne = e_idx.size
            assert ne <= TW * 128, f"window overflow: core {c} win {w}: {ne}"
            base = w * TW * 128
            snd_c[base:base + ne] = senders[e_idx]
            rcv_c[base:base + ne] = col_of[receivers[e_idx]].astype(np.float32)
            vec_c[base:base + ne] = vectors[e_idx]
            rad_c[base:base + ne] = radial[e_idx]
        # host-built one-hot scatter matrices: oh[p, t*128 + j] =
        # (col[t*128+p] == j); pads (sentinel 192) never match.
        oh = (rcv_c.reshape(TC, 128).T[:, :, None]
              == np.arange(128, dtype=np.float32)[None, None, :])
        in_maps.append({
            "nf": nf_g,
            "snd": np.ascontiguousarray(
                snd_c.reshape(TC, 128).T.astype(np.int32)),
            "ohm": np.ascontiguousarray(oh.reshape(128, TC * 128)).astype(nbf16),
            "vec": np.ascontiguousarray(
                vec_c.reshape(TC, 128, 3).transpose(1, 0, 2).reshape(128, TC * 3)),
            "rad": np.ascontiguousarray(rad_c.T).astype(nbf16),
            "w1": W_rad1.astype(nbf16),
            "b1": b_rad1[:, None].astype(np.float32),
            "w2": w2lf,
            "wq": wq,
            "c2w": node_layout(c2n, c),
            "c3w": node_layout(c3n, c),
            "nf0": node_layout(nf0, c),
            "usc": node_layout(U, c),
        })
    return in_maps, win_of, col_of


def assemble_output(results, win_of, col_of):
    """results: list of 8 dicts with 'out' [128, WPC] -> [N_NODES, 1] f32."""
    full = np.zeros((N_NODES,), np.float32)
    outs = np.stack([np.asarray(results[c]["out"], np.float32)
                     for c in range(N_CORES)])  # [8, 128, WPC]
    full = outs[win_of // WPC, col_of, win_of % WPC]
    return full[:, None].copy()


_CACHED_NC = None
LAST_EXEC_NS = None
LAST_RESULTS = None


def kernel(**inputs):
    global _CACHED_NC, LAST_EXEC_NS, LAST_RESULTS
    from concourse.bass_utils import run_bass_kernel_spmd
    in_maps, win_of, col_of = host_prep(inputs)
    if _CACHED_NC is None:
        _CACHED_NC = build_program()
    trace = bool(int(os.environ.get("MACE_TRACE", "0")))
    kwargs = {}
    if trace:
        kwargs.update(trace=True, trace_cores=[0], tmpdir="/root/problem/trace_out")
        os.makedirs("/root/problem/trace_out", exist_ok=True)
    res = run_bass_kernel_spmd(_CACHED_NC, in_maps, list(range(N_CORES)), **kwargs)
    LAST_EXEC_NS = res.exec_time_ns
    LAST_RESULTS = res
    return assemble_output(res.results, win_of, col_of)
